# revision 1
# baseline (speedup 1.0000x reference)
"""Self-attention (nn_Attention_85169201480320) as a distributed Bass kernel
on 8 TRN2 NeuronCores.

Reference computation (B=2, S=2048, H=1024, NH=16, HD=64):
    mixed_query = x @ Wq + bq
    query = split_heads(mixed_query @ Wq + bq)     # double-apply bug preserved
    key   = split_heads(x @ Wk + bk)
    value = split_heads(x @ Wv + bv)
    out   = softmax(q k^T / sqrt(HD)) v            # per (batch, head)

Sharding: tensor-parallel over heads — core c owns heads {2c, 2c+1}, i.e.
columns [c*128, (c+1)*128) of the QKV projections and of the output. x is
replicated (pre-transposed and cast to fp16 on host). The double Q
projection is folded on the host: query = x @ (Wq@Wq) + (bq@Wq + bq).

Per-core device graph (no collectives needed):
  - Q^T, K^T in [d, seq] layout: psum = W_chunk^T-stationary @ x^T-moving,
    bias added on VectorE during the PSUM->SBUF copy.
  - V in [seq, d] layout (stationary = x^T chunk, moving = Wv slice), with
    a constant ones column appended (column 64) so the PV matmul also
    computes the softmax row-sums. V bias is deferred to the epilogue
    (softmax rows sum to 1).
  - Attention per (batch b, q-chunk of 512, k-chunk of 128):
      S^T[k, q] for both heads via row-tiled (64-contraction) matmuls into
      one [128, 1024] PSUM tile; one ScalarE Exp (scale=1/8 folded) into an
      fp16 E^T tile; PV matmuls accumulate [65, 512] per head over k-chunks.
  - Epilogue: PE-transpose of [65, 128] C^T tiles -> [128, 65], reciprocal
    of the sumexp column, out = C*recip + bv broadcast, DMA out.
"""

import numpy as np

B, S, H = 2, 2048, 1024
NH, HD = 16, 64
N_CORES = 8
HPC = NH // N_CORES        # heads per core = 2
CPC = HPC * HD             # output columns per core = 128
BS = B * S                 # 4096 rows total
SCALE = HD ** -0.5
# spare knob: a constant folded out of exp (cancels in the final division
# by the identically-scaled row sums); currently 0 in the exp call below.
EXP_BIAS = 2.25

HC = H // 128              # 8 contraction chunks
SC_ALL = BS // 128         # 32 seq chunks of 128
SC_B = S // 128            # 16 seq chunks per batch
QC_B = S // 512            # 4 q-chunks of 512 per batch
QT_B = S // 128            # 16 q-tiles of 128 per batch


def build_kernel(repeat: int = 1, variant: str = 'bg'):
    import concourse.bass as bass
    import concourse.mybir as mybir
    import concourse.tile as tile
    from concourse import bacc
    from concourse.masks import make_identity

    fp16 = mybir.dt.float16
    f32 = mybir.dt.float32

    nc = bacc.Bacc("TRN2", target_bir_lowering=False, debug=False,
                   num_devices=N_CORES)

    xT = nc.declare_dram_parameter("xT", [H, BS], fp16, isOutput=False)
    wq = nc.declare_dram_parameter("wq", [H, CPC], fp16, isOutput=False)
    wk = nc.declare_dram_parameter("wk", [H, CPC], fp16, isOutput=False)
    wv = nc.declare_dram_parameter("wv", [H, CPC], fp16, isOutput=False)
    bq = nc.declare_dram_parameter("bq", [CPC, 1], f32, isOutput=False)
    bk = nc.declare_dram_parameter("bk", [CPC, 1], f32, isOutput=False)
    bv = nc.declare_dram_parameter("bv", [CPC], f32, isOutput=False)
    out = nc.declare_dram_parameter("out", [BS, CPC], f32, isOutput=True)

    with tile.TileContext(nc) as tc:
        with (
            tc.tile_pool(name="big", bufs=1) as big,
            tc.tile_pool(name="work", bufs=2) as work,
            tc.tile_pool(name="psum", bufs=1, space="PSUM") as psum,
        ):
            # ---- constants / small inputs ----
            ident = big.tile([65, 65], f32)
            make_identity(nc, ident)
            bq_sb = big.tile([CPC, 1], f32)
            nc.sync.dma_start(out=bq_sb, in_=bq[:, :])
            bk_sb = big.tile([CPC, 1], f32)
            nc.sync.dma_start(out=bk_sb, in_=bk[:, :])
            # bv broadcast to all 128 partitions: [128, 128]
            bv_sb = big.tile([128, CPC], f32)
            expb_sb = big.tile([128, 1], f32)
            nc.vector.memset(expb_sb, -EXP_BIAS)
            bv_ap = bv.ap()
            bv_bcast = bass.AP(tensor=bv_ap.tensor, offset=bv_ap.offset,
                               ap=[[0, 128], [1, CPC]])
            nc.gpsimd.dma_start(out=bv_sb, in_=bv_bcast)

            # ---- big persistent SBUF tensors ----
            # weights first: the first projection matmuls need w + one xT
            # chunk, so don't queue 8MB of xT DMA ahead of them.
            w_sb = big.tile([128, HC, 3, CPC], fp16)       # 6KB/part
            # one strided DMA per weight tensor: [1024,128] -> [128, hc, 128]
            for t, w in ((0, wq), (1, wk), (2, wv)):
                nc.sync.dma_start(
                    out=w_sb[:, :, t, :],
                    in_=w.ap().rearrange("(c p) m -> p c m", p=128))
            xT_sb = big.tile([128, HC, BS], fp16)          # 64KB/part
            # on the SP ring (NOT the ACT ring: the ACT sequencer must be
            # free to issue the first exp the moment S^T(0) lands -- DMA
            # issues cost ~0.7-2.2us each on the issuing sequencer).
            # Order: the first 512 seq-columns of every hc chunk land first
            # (that is all K/Q chunk 0 needs, so the exp stream can start
            # ~10us earlier), then the rest of batch 0, then batch 1.
            for hc in range(HC):
                nc.sync.dma_start(
                    out=xT_sb[:, hc, 0:512],
                    in_=xT[hc * 128:(hc + 1) * 128, 0:512])
            for hc in range(HC):
                nc.sync.dma_start(
                    out=xT_sb[:, hc, 512:S],
                    in_=xT[hc * 128:(hc + 1) * 128, 512:S])
            for hc in range(HC):
                nc.sync.dma_start(
                    out=xT_sb[:, hc, S:BS],
                    in_=xT[hc * 128:(hc + 1) * 128, S:BS])

            qkT = big.tile([128, 2, BS], fp16)             # 16KB/part
            v_sb = big.tile([128, HPC, SC_ALL, 65], fp16)  # 8.3KB/part
            cuT = big.tile([65, 2 * HPC, S], f32)          # 32KB/part

            # ones column of V_aug (written once; V copies touch only 0:64)
            nc.vector.memset(v_sb[:, :, :, 64:65], 1.0)

            def emit_body():
                _emit_attention_body(nc, tc, bass, mybir, psum, work, big,
                                     xT_sb, w_sb, qkT, v_sb, cuT,
                                     bq_sb, bk_sb, bv_sb, ident, out, expb_sb,
                                     variant)

            if repeat == 1:
                emit_body()
            else:
                with tc.For_i(0, repeat, 1):
                    emit_body()

    nc.finalize()
    return nc


def _emit_attention_body(nc, tc, bass, mybir, psum, work, big,
                         xT_sb, w_sb, qkT, v_sb, cuT,
                         bq_sb, bk_sb, bv_sb, ident, out, expb_sb,
                         variant='bg'):
            fp16 = mybir.dt.float16
            f32 = mybir.dt.float32
            # ---- emission plan ----
            # attention chunk (b, qc) needs: Q chunk sc=4b+qc, ALL of K for
            # batch b, and V chunks racing ahead of its kc loop. So: project
            # K(b0) + Q(b0,sc0) first (hc-outer, so the PE starts on the
            # first 512KB xT DMA), start attention immediately, and feed the
            # remaining Q/K/V projections in as fillers between (and inside)
            # attention chunks, where they soak up PE slack under the
            # ScalarE-paced exp stream.
            def emit_proj_hc_outer(jobs):
                # jobs: list of (t, sc, tag) -> one [128,512] psum tile each
                tiles = [
                    psum.tile([128, 512], f32, tag=tag, bufs=2,
                              name=f"pj0_{t}_{sc}")
                    for t, sc, tag in jobs
                ]
                for hc in range(HC):
                    for (t, sc, _), ps in zip(jobs, tiles):
                        nc.tensor.matmul(
                            ps,
                            w_sb[:, hc, t, :],
                            xT_sb[:, hc, sc * 512:(sc + 1) * 512],
                            start=(hc == 0), stop=(hc == HC - 1),
                        )
                for (t, sc, _), ps in zip(jobs, tiles):
                    nc.vector.tensor_scalar_add(
                        qkT[:, t, sc * 512:(sc + 1) * 512], ps,
                        bq_sb if t == 0 else bk_sb,
                    )

            def emit_proj(t, sc):
                ps = psum.tile([128, 512], f32, tag="aux", bufs=2,
                               name=f"pj_{t}_{sc}")
                for hc in range(HC):
                    nc.tensor.matmul(
                        ps,
                        w_sb[:, hc, t, :],
                        xT_sb[:, hc, sc * 512:(sc + 1) * 512],
                        start=(hc == 0), stop=(hc == HC - 1),
                    )
                nc.vector.tensor_scalar_add(
                    qkT[:, t, sc * 512:(sc + 1) * 512], ps,
                    bq_sb if t == 0 else bk_sb,
                )

            def emit_v_chunk(sc):
                ps = psum.tile([128, CPC], f32, tag="aux", bufs=2,
                               name=f"psv_{sc}")
                for hc in range(HC):
                    nc.tensor.matmul(
                        ps,
                        xT_sb[:, hc, sc * 128:(sc + 1) * 128],
                        w_sb[:, hc, 2, :],
                        start=(hc == 0), stop=(hc == HC - 1),
                    )
                # [128, 2, 64] strided copy into v_sb (both heads)
                nc.vector.tensor_copy(
                    v_sb[:, :, sc, 0:64],
                    ps.rearrange("p (h d) -> p h d", h=HPC),
                )

            V = lambda s: (lambda: emit_v_chunk(s))
            P = lambda t, s: (lambda: emit_proj(t, s))

            if variant == 'bgpaced':
                # like 'bg', but gives the scheduler a pacing hint per
                # background piece (earliest useful time, us) so it does not
                # front-stuff V work ahead of the first exp stream.
                emit_proj_hc_outer([(1, 0, "sT"), (1, 1, "sT"),
                                    (1, 2, "pv"), (1, 3, "pv"),
                                    (0, 0, "aux"), (0, 1, "aux")])
                with tc.high_priority(offset=-1_000_000):
                    def at(us, f):
                        with tc.tile_wait_until(us / 1000.0):
                            f()
                    for sc in range(SC_B):
                        at(10 + sc * 1.2, lambda s=sc: emit_v_chunk(s))
                    at(28, lambda: emit_proj(0, 2))
                    at(38, lambda: emit_proj(0, 3))
                    for i, sc in enumerate(range(4, HC)):
                        at(45 + 6 * i, lambda s=sc: emit_proj(1, s))
                    for sc in range(SC_B, SC_ALL):
                        at(55 + (sc - SC_B) * 1.2, lambda s=sc: emit_v_chunk(s))
                    at(72, lambda: emit_proj(0, 4))
                    at(88, lambda: emit_proj(0, 5))
                    at(107, lambda: emit_proj(0, 6))
                    at(126, lambda: emit_proj(0, 7))
                mid_fill = {}
                end_fill = {(b, qc): [] for b in range(B)
                            for qc in range(QC_B)}
            elif variant == 'bg':
                # K(b0) + the first two Q chunks up front (hc-outer so the
                # PE tracks the xT DMA); everything else -- remaining Q/K
                # projections and all V chunks -- is emitted ONCE at
                # background priority, in rough consumption order. The Tile
                # scheduler then runs it in PE idle slots, and data
                # dependencies pull each piece in just-in-time.
                emit_proj_hc_outer([(1, 0, "sT"), (0, 0, "aux")])
                emit_proj_hc_outer([(1, 1, "sT"), (1, 2, "pv"),
                                    (1, 3, "pv"), (0, 1, "aux")])
                for sc in range(8):
                    emit_v_chunk(sc)
                with tc.high_priority(offset=-1_000_000):
                    for sc in range(8, SC_B):
                        emit_v_chunk(sc)
                    for sc in range(4, HC):
                        emit_proj(1, sc)
                    emit_proj(0, 2)
                    emit_proj(0, 3)
                    for sc in range(SC_B, SC_ALL):
                        emit_v_chunk(sc)
                    for sc in range(4, HC):
                        emit_proj(0, sc)
                mid_fill = {}
                end_fill = {(b, qc): [] for b in range(B)
                            for qc in range(QC_B)}
            elif variant == 'midfill':
                # startup: K(b0) fully (every attention chunk of b0 needs
                # all of K), Q chunk 0, and the first V chunks. Everything
                # else fills PE slack inside attention chunks via mid_fill:
                # mid_fill[(b,qc)][kc] = thunks after that kc iteration,
                # paced ~1 V chunk (or 1/2 proj tile) per iteration, with a
                # >=3-iteration lead on the consuming PV.
                emit_proj_hc_outer([(1, 0, "sT"), (1, 1, "sT"),
                                    (1, 2, "pv"), (1, 3, "pv"),
                                    (0, 0, "aux")])
                for sc in range(4):
                    emit_v_chunk(sc)
                mid_fill = {
                    (0, 0): {**{kc: [V(3 + kc)] for kc in range(1, 13)},
                             13: [P(0, 1)]},
                    (0, 1): {2: [P(1, 4)], 7: [P(0, 2)], 12: [P(1, 5)]},
                    (0, 2): {2: [P(1, 6)], 7: [P(0, 3)], 12: [P(1, 7)]},
                    (0, 3): {**{kc: [V(14 + kc)] for kc in range(2, 10)},
                             11: [P(0, 4)]},
                    (1, 0): {**{kc: [V(23 + kc)] for kc in range(1, 9)}},
                }
                end_fill = {
                    (0, 0): [], (0, 1): [], (0, 2): [], (0, 3): [],
                    (1, 0): [P(0, 5)], (1, 1): [P(0, 6)],
                    (1, 2): [P(0, 7)], (1, 3): [],
                }
            else:  # 'upfront'
                emit_proj_hc_outer([(1, 0, "sT"), (1, 1, "sT"),
                                    (1, 2, "pv"), (1, 3, "pv"),
                                    (0, 0, "aux"), (0, 1, "aux")])
                for sc in range(SC_B):
                    emit_v_chunk(sc)

                def b1_slice(i):
                    t = i % 2
                    sc = 4 + 2 * (i // 2)
                    for s in (sc, sc + 1):
                        emit_proj(t, s)
                    for s in range(SC_B + 4 * i, SC_B + 4 * i + 4):
                        emit_v_chunk(s)

                mid_fill = {}
                end_fill = {
                    (0, 0): [P(0, 2), lambda: b1_slice(0)],
                    (0, 1): [P(0, 3), lambda: b1_slice(1)],
                    (0, 2): [lambda: b1_slice(2)],
                    (0, 3): [lambda: b1_slice(3)],
                    (1, 0): [], (1, 1): [], (1, 2): [], (1, 3): [],
                }

            # ---- attention ----
            for b in range(B):
                for qc in range(QC_B):  # q-chunks of 512
                    q0 = b * S + qc * 512
                    pv = [
                        psum.tile([65, 512], f32, tag="pv", bufs=2,
                                  name=f"pv_{b}_{qc}_{h}")
                        for h in range(HPC)
                    ]
                    # kc loop, software-pipelined: PV trails one iteration
                    # so the PE always issues the next S^T (which feeds the
                    # ScalarE exp stream, the pacer) before the current PV.
                    eTs = {}

                    def emit_pv(kc):
                        eT_prev = eTs.pop(kc)
                        for h in range(HPC):
                            nc.tensor.matmul(
                                pv[h],
                                v_sb[:, h, b * SC_B + kc, :],
                                eT_prev[:, h * 512:(h + 1) * 512],
                                start=(kc == 0), stop=(kc == SC_B - 1),
                            )

                    for kc in range(SC_B):  # k-chunks of 128
                        k0 = b * S + kc * 128
                        sT = psum.tile([128, 1024], f32, tag="sT", bufs=2,
                                       name=f"sT_{b}_{qc}_{kc}")
                        for h in range(HPC):
                            r0, r1 = h * 64, (h + 1) * 64
                            nc.tensor.matmul(
                                sT[:, h * 512:(h + 1) * 512],
                                qkT[r0:r1, 1, k0:k0 + 128],
                                qkT[r0:r1, 0, q0:q0 + 512],
                                start=True, stop=True,
                            )
                        eT = work.tile([128, 1024], fp16, tag="eT", bufs=3,
                                       name=f"eT_{b}_{qc}_{kc}")
                        nc.scalar.activation(
                            eT, sT, mybir.ActivationFunctionType.Exp,
                            bias=0.0, scale=SCALE,
                        )
                        eTs[kc] = eT
                        if kc > 0:
                            emit_pv(kc - 1)
                        for f in mid_fill.get((b, qc), {}).get(kc, ()):
                            f()
                    emit_pv(SC_B - 1)
                    for h in range(HPC):
                        nc.vector.tensor_copy(
                            cuT[:, 2 * b + h, qc * 512:(qc + 1) * 512], pv[h]
                        )

                    # epilogue for this chunk: transpose, normalize, bias,
                    # store. Mid-band priority: deferred into PE idle slots,
                    # but ahead of the projection/V background.
                    ep_ctx = tc.high_priority(offset=-500_000) \
                        if variant in ('bg', 'bgpaced') else None
                    if ep_ctx is not None:
                        ep_ctx.__enter__()
                    for qt in range(qc * 4, qc * 4 + 4):  # q-tiles of 128
                        o_sb = work.tile([128, CPC], f32, tag="osb", bufs=3,
                                         name=f"osb_{b}_{qt}")
                        for h in range(HPC):
                            tr = psum.tile([128, 65], f32, tag="aux", bufs=2,
                                           name=f"tr_{b}_{qt}_{h}")
                            nc.tensor.transpose(
                                tr, cuT[:, 2 * b + h, qt * 128:(qt + 1) * 128],
                                ident,
                            )
                            rec = work.tile([128, 1], f32, tag="rec", bufs=4,
                                            name=f"rec_{b}_{qt}_{h}")
                            nc.vector.reciprocal(rec, tr[:, 64:65])
                            nc.vector.scalar_tensor_tensor(
                                o_sb[:, h * 64:(h + 1) * 64],
                                tr[:, 0:64], rec, bv_sb[:, h * 64:(h + 1) * 64],
                                op0=mybir.AluOpType.mult,
                                op1=mybir.AluOpType.add,
                            )
                        r0 = b * S + qt * 128
                        nc.sync.dma_start(out=out[r0:r0 + 128, :], in_=o_sb)
                    if ep_ctx is not None:
                        ep_ctx.__exit__(None, None, None)
                    for f in end_fill[(b, qc)]:
                        f()


def prep_inputs(x, Wq, bq, Wk, bk, Wv, bv):
    """Host-side prep: fold the double Q projection, transpose/cast x,
    slice per-core weights."""
    x = np.asarray(x, np.float32)
    Wq = np.asarray(Wq, np.float64)
    bq = np.asarray(bq, np.float64)
    Wq2 = (Wq @ Wq).astype(np.float32)
    bq2 = (bq @ Wq + bq).astype(np.float32)
    Wk = np.asarray(Wk, np.float32)
    Wv = np.asarray(Wv, np.float32)
    bk = np.asarray(bk, np.float32)
    bv = np.asarray(bv, np.float32)

    xT = np.ascontiguousarray(x.reshape(BS, H).T).astype(np.float16)

    in_maps = []
    for c in range(N_CORES):
        lo, hi = c * CPC, (c + 1) * CPC
        in_maps.append({
            "xT": xT,
            "wq": np.ascontiguousarray(Wq2[:, lo:hi]).astype(np.float16),
            "wk": np.ascontiguousarray(Wk[:, lo:hi]).astype(np.float16),
            "wv": np.ascontiguousarray(Wv[:, lo:hi]).astype(np.float16),
            "bq": np.ascontiguousarray(bq2[lo:hi]).reshape(CPC, 1),
            "bk": np.ascontiguousarray(bk[lo:hi]).reshape(CPC, 1),
            "bv": np.ascontiguousarray(bv[lo:hi]),
        })
    return in_maps


_CACHED = {}


def kernel(x, Wq, bq, Wk, bk, Wv, bv):
    from concourse.bass_utils import run_bass_kernel_spmd

    if "nc" not in _CACHED:
        _CACHED["nc"] = build_kernel()
    nc = _CACHED["nc"]

    in_maps = prep_inputs(x, Wq, bq, Wk, bk, Wv, bv)
    res = run_bass_kernel_spmd(nc, in_maps, core_ids=list(range(N_CORES)))

    full = np.empty((BS, NH * HD), np.float32)
    for c in range(N_CORES):
        full[:, c * CPC:(c + 1) * CPC] = res.results[c]["out"]
    return full.reshape(B, S, NH * HD)


if __name__ == "__main__":
    nc = build_kernel()
    print("built ok")



# revision 21
# speedup vs baseline: 2.3336x; 2.3336x over previous
"""Self-attention (nn_Attention_85169201480320) as a distributed Bass kernel
on 8 TRN2 NeuronCores.

Reference computation (B=2, S=2048, H=1024, NH=16, HD=64):
    mixed_query = x @ Wq + bq
    query = split_heads(mixed_query @ Wq + bq)     # double-apply bug preserved
    key   = split_heads(x @ Wk + bk)
    value = split_heads(x @ Wv + bv)
    out   = softmax(q k^T / sqrt(HD)) v            # per (batch, head)

Sharding: tensor-parallel over heads — core c owns heads {2c, 2c+1}, i.e.
columns [c*128, (c+1)*128) of the QKV projections and of the output. x is
replicated (pre-transposed and cast to fp16 on host). The double Q
projection is folded on the host: query = x @ (Wq@Wq) + (bq@Wq + bq).

Per-core device graph (no collectives needed):
  - Q^T, K^T in [d, seq] layout: psum = W_chunk^T-stationary @ x^T-moving,
    bias added on VectorE during the PSUM->SBUF copy.
  - V in [seq, d] layout (stationary = x^T chunk, moving = Wv slice), with
    a constant ones column appended (column 64) so the PV matmul also
    computes the softmax row-sums. V bias is deferred to the epilogue
    (softmax rows sum to 1).
  - Attention per (batch b, q-chunk of 512, k-chunk of 128):
      S^T[k, q] for both heads via row-tiled (64-contraction) matmuls into
      one [128, 1024] PSUM tile; one ScalarE Exp (scale=1/8 folded) into an
      fp16 E^T tile; PV matmuls accumulate [65, 512] per head over k-chunks.
  - Epilogue: PE-transpose of [65, 128] C^T tiles -> [128, 65], reciprocal
    of the sumexp column, out = C*recip + bv broadcast, DMA out.
"""

import numpy as np

B, S, H = 2, 2048, 1024
NH, HD = 16, 64
N_CORES = 8
HPC = NH // N_CORES        # heads per core = 2
CPC = HPC * HD             # output columns per core = 128
BS = B * S                 # 4096 rows total
SCALE = HD ** -0.5
# spare knob: a constant folded out of exp (cancels in the final division
# by the identically-scaled row sums); currently 0 in the exp call below.
EXP_BIAS = 2.25

HC = H // 128              # 8 contraction chunks
SC_ALL = BS // 128         # 32 seq chunks of 128
SC_B = S // 128            # 16 seq chunks per batch
QC_B = S // 512            # 4 q-chunks of 512 per batch
QT_B = S // 128            # 16 q-tiles of 128 per batch


def build_kernel(repeat: int = 1, variant: str = 'v4'):
    import concourse.bass as bass
    import concourse.mybir as mybir
    import concourse.tile as tile
    from concourse import bacc
    from concourse.masks import make_identity

    fp16 = mybir.dt.float16
    f32 = mybir.dt.float32

    nc = bacc.Bacc("TRN2", target_bir_lowering=False, debug=False,
                   num_devices=N_CORES)

    xT = nc.declare_dram_parameter("xT", [H, BS], fp16, isOutput=False)
    wq = nc.declare_dram_parameter("wq", [H, CPC], fp16, isOutput=False)
    wk = nc.declare_dram_parameter("wk", [H, CPC], fp16, isOutput=False)
    wv = nc.declare_dram_parameter("wv", [H, CPC], fp16, isOutput=False)
    bq = nc.declare_dram_parameter("bq", [CPC, 1], f32, isOutput=False)
    bk = nc.declare_dram_parameter("bk", [CPC, 1], f32, isOutput=False)
    bv = nc.declare_dram_parameter("bv", [CPC], f32, isOutput=False)
    out = nc.declare_dram_parameter("out", [BS, CPC], f32, isOutput=True)

    with tile.TileContext(nc) as tc:
        with (
            tc.tile_pool(name="big", bufs=1) as big,
            tc.tile_pool(name="work", bufs=2) as work,
            tc.tile_pool(name="psum", bufs=1, space="PSUM") as psum,
        ):
            # ---- constants / small inputs ----
            ident = big.tile([65, 65], f32)
            make_identity(nc, ident)
            bq_sb = big.tile([CPC, 1], f32)
            bk_sb = big.tile([CPC, 1], f32)
            # bv broadcast to all 128 partitions: [128, 128]
            bv_sb = big.tile([128, CPC], f32)
            expb_sb = big.tile([128, 1], f32)
            nc.vector.memset(expb_sb, -EXP_BIAS)
            bv_ap = bv.ap()
            bv_bcast = bass.AP(tensor=bv_ap.tensor, offset=bv_ap.offset,
                               ap=[[0, 128], [1, CPC]])
            if not variant.startswith(('v3', 'v4')):
                nc.sync.dma_start(out=bq_sb, in_=bq[:, :])
                nc.sync.dma_start(out=bk_sb, in_=bk[:, :])
                nc.gpsimd.dma_start(out=bv_sb, in_=bv_bcast)

            # ---- big persistent SBUF tensors ----
            # weights first: the first projection matmuls need w + one xT
            # chunk, so don't queue 8MB of xT DMA ahead of them.
            w_sb = big.tile([128, HC, 3, CPC], fp16)       # 6KB/part
            xT_sb = big.tile([128, HC, BS], fp16)          # 64KB/part
            if variant.startswith(('v3', 'v4')):
                # DMA issue costs ~650ns on the issuing sequencer, so the
                # serial-on-SP baseline pays ~19us before the first
                # projection data is even queued. Spread issues over four
                # engine rings (SP, Pool, DVE, PE -- ACT stays free for the
                # exp stream) and use 2-hc strided chunks so the first 512
                # seq-columns of every hc land within ~2.5us.
                # wq/wk first (block the first projection), then the first
                # 512 seq-cols of every hc, then wv (first V fill ~10us),
                # then the rest in consumption order. bv does not matter
                # until the first epilogue (~30us) -- it goes last.
                for t, w, eng in ((0, wq, nc.gpsimd), (1, wk, nc.scalar)):
                    eng.dma_start(
                        out=w_sb[:, :, t, :],
                        in_=w.ap().rearrange("(c p) m -> p c m", p=128))
                ph0 = (nc.sync, nc.gpsimd, nc.scalar, nc.sync)
                for i in range(4):
                    ph0[i].dma_start(
                        out=xT_sb[:, 2 * i:2 * i + 2, 0:512],
                        in_=xT[256 * i:256 * i + 256, 0:512].rearrange(
                            "(c p) m -> p c m", p=128))
                nc.gpsimd.dma_start(
                    out=w_sb[:, :, 2, :],
                    in_=wv.ap().rearrange("(c p) m -> p c m", p=128))
                nc.scalar.dma_start(out=bq_sb, in_=bq[:, :])
                nc.scalar.dma_start(out=bk_sb, in_=bk[:, :])
                for c0, c1 in ((512, 1024), (1024, 1536), (1536, 2048),
                               (2048, 3072), (3072, 4096)):
                    for i in range(4):
                        (nc.sync if i % 2 == 0 else nc.gpsimd).dma_start(
                            out=xT_sb[:, 2 * i:2 * i + 2, c0:c1],
                            in_=xT[256 * i:256 * i + 256, c0:c1].rearrange(
                                "(c p) m -> p c m", p=128))
                nc.gpsimd.dma_start(out=bv_sb, in_=bv_bcast)
            else:
                # one strided DMA per weight tensor:
                # [1024,128] -> [128, hc, 128]
                for t, w in ((0, wq), (1, wk), (2, wv)):
                    nc.sync.dma_start(
                        out=w_sb[:, :, t, :],
                        in_=w.ap().rearrange("(c p) m -> p c m", p=128))
                # on the SP ring (NOT the ACT ring: the ACT sequencer must be
                # free to issue the first exp the moment S^T(0) lands -- DMA
                # issues cost ~0.7-2.2us each on the issuing sequencer).
                # Order: the first 512 seq-columns of every hc chunk land
                # first (that is all K/Q chunk 0 needs, so the exp stream can
                # start ~10us earlier), then the rest of batch 0, then b1.
                for hc in range(HC):
                    nc.sync.dma_start(
                        out=xT_sb[:, hc, 0:512],
                        in_=xT[hc * 128:(hc + 1) * 128, 0:512])
                for hc in range(HC):
                    nc.sync.dma_start(
                        out=xT_sb[:, hc, 512:S],
                        in_=xT[hc * 128:(hc + 1) * 128, 512:S])
                for hc in range(HC):
                    nc.sync.dma_start(
                        out=xT_sb[:, hc, S:BS],
                        in_=xT[hc * 128:(hc + 1) * 128, S:BS])

            qkT = big.tile([128, 2, BS], fp16)             # 16KB/part
            v_sb = big.tile([128, HPC, SC_ALL, 65], fp16)  # 8.3KB/part
            cuT = big.tile([65, 2 * HPC, S], f32)          # 32KB/part

            # ones column of V_aug (written once; V copies touch only 0:64)
            nc.vector.memset(v_sb[:, :, :, 64:65], 1.0)

            if variant.startswith(('v3', 'v4')):
                # dummy exp: pulls the ~2.7us ACT table load into the DMA
                # phase, where the ACT engine is otherwise idle.
                dmy = big.tile([1, 2], f32)
                nc.vector.memset(dmy, 0.0)
                dmy_o = big.tile([1, 2], fp16)
                nc.scalar.activation(
                    dmy_o, dmy, mybir.ActivationFunctionType.Exp,
                    bias=0.0, scale=1.0)

            def emit_body():
                if variant.startswith('v4'):
                    _emit_v4_body(nc, tc, bass, mybir, psum, work, big,
                                  xT_sb, w_sb, qkT, v_sb, cuT,
                                  bq_sb, bk_sb, bv_sb, ident, out)
                else:
                    _emit_attention_body(nc, tc, bass, mybir, psum, work,
                                         big, xT_sb, w_sb, qkT, v_sb, cuT,
                                         bq_sb, bk_sb, bv_sb, ident, out,
                                         expb_sb, variant)

            if repeat == 1:
                emit_body()
            else:
                with tc.For_i(0, repeat, 1):
                    emit_body()

    nc.finalize()
    return nc


def _emit_attention_body(nc, tc, bass, mybir, psum, work, big,
                         xT_sb, w_sb, qkT, v_sb, cuT,
                         bq_sb, bk_sb, bv_sb, ident, out, expb_sb,
                         variant='bg'):
            fp16 = mybir.dt.float16
            f32 = mybir.dt.float32
            # ---- emission plan ----
            # attention chunk (b, qc) needs: Q chunk sc=4b+qc, ALL of K for
            # batch b, and V chunks racing ahead of its kc loop. So: project
            # K(b0) + Q(b0,sc0) first (hc-outer, so the PE starts on the
            # first 512KB xT DMA), start attention immediately, and feed the
            # remaining Q/K/V projections in as fillers between (and inside)
            # attention chunks, where they soak up PE slack under the
            # ScalarE-paced exp stream.
            def emit_proj_hc_outer(jobs):
                # jobs: list of (t, sc, tag) -> one [128,512] psum tile each
                tiles = [
                    psum.tile([128, 512], f32, tag=tag, bufs=2,
                              name=f"pj0_{t}_{sc}")
                    for t, sc, tag in jobs
                ]
                for hc in range(HC):
                    for (t, sc, _), ps in zip(jobs, tiles):
                        nc.tensor.matmul(
                            ps,
                            w_sb[:, hc, t, :],
                            xT_sb[:, hc, sc * 512:(sc + 1) * 512],
                            start=(hc == 0), stop=(hc == HC - 1),
                        )
                for (t, sc, _), ps in zip(jobs, tiles):
                    nc.vector.tensor_scalar_add(
                        qkT[:, t, sc * 512:(sc + 1) * 512], ps,
                        bq_sb if t == 0 else bk_sb,
                    )

            def emit_proj(t, sc):
                ps = psum.tile([128, 512], f32, tag="aux", bufs=2,
                               name=f"pj_{t}_{sc}")
                for hc in range(HC):
                    nc.tensor.matmul(
                        ps,
                        w_sb[:, hc, t, :],
                        xT_sb[:, hc, sc * 512:(sc + 1) * 512],
                        start=(hc == 0), stop=(hc == HC - 1),
                    )
                nc.vector.tensor_scalar_add(
                    qkT[:, t, sc * 512:(sc + 1) * 512], ps,
                    bq_sb if t == 0 else bk_sb,
                )

            def emit_v_chunk(sc):
                ps = psum.tile([128, CPC], f32, tag="aux", bufs=2,
                               name=f"psv_{sc}")
                for hc in range(HC):
                    nc.tensor.matmul(
                        ps,
                        xT_sb[:, hc, sc * 128:(sc + 1) * 128],
                        w_sb[:, hc, 2, :],
                        start=(hc == 0), stop=(hc == HC - 1),
                    )
                # [128, 2, 64] strided copy into v_sb (both heads)
                nc.vector.tensor_copy(
                    v_sb[:, :, sc, 0:64],
                    ps.rearrange("p (h d) -> p h d", h=HPC),
                )

            V = lambda s: (lambda: emit_v_chunk(s))
            P = lambda t, s: (lambda: emit_proj(t, s))

            if variant == 'bgpaced':
                # like 'bg', but gives the scheduler a pacing hint per
                # background piece (earliest useful time, us) so it does not
                # front-stuff V work ahead of the first exp stream.
                emit_proj_hc_outer([(1, 0, "sT"), (1, 1, "sT"),
                                    (1, 2, "pv"), (1, 3, "pv"),
                                    (0, 0, "aux"), (0, 1, "aux")])
                with tc.high_priority(offset=-1_000_000):
                    def at(us, f):
                        with tc.tile_wait_until(us / 1000.0):
                            f()
                    for sc in range(SC_B):
                        at(10 + sc * 1.2, lambda s=sc: emit_v_chunk(s))
                    at(28, lambda: emit_proj(0, 2))
                    at(38, lambda: emit_proj(0, 3))
                    for i, sc in enumerate(range(4, HC)):
                        at(45 + 6 * i, lambda s=sc: emit_proj(1, s))
                    for sc in range(SC_B, SC_ALL):
                        at(55 + (sc - SC_B) * 1.2, lambda s=sc: emit_v_chunk(s))
                    at(72, lambda: emit_proj(0, 4))
                    at(88, lambda: emit_proj(0, 5))
                    at(107, lambda: emit_proj(0, 6))
                    at(126, lambda: emit_proj(0, 7))
                mid_fill = {}
                end_fill = {(b, qc): [] for b in range(B)
                            for qc in range(QC_B)}
            elif variant.startswith('v3'):
                # K(b0) sc0 + Q(b0) sc0 foreground (hc-outer, DMA-paced);
                # everything else background in exact consumption order so
                # the greedy scheduler's ready-queue matches the exp
                # stream's needs and data-arrival order.
                emit_proj_hc_outer([(1, 0, "sT"), (0, 0, "aux")])
                with tc.high_priority(offset=-1_000_000):
                    emit_proj(1, 1)
                    for sc in range(0, 4):
                        emit_v_chunk(sc)
                    emit_proj(1, 2)
                    for sc in range(4, 8):
                        emit_v_chunk(sc)
                    emit_proj(1, 3)
                    for sc in range(8, 12):
                        emit_v_chunk(sc)
                    emit_proj(0, 1)
                    for sc in range(12, 16):
                        emit_v_chunk(sc)
                    emit_proj(0, 2)
                    emit_proj(0, 3)
                    for sc in range(4, HC):   # K(b1)
                        emit_proj(1, sc)
                    emit_proj(0, 4)           # Q(b1, qc0)
                    for sc in range(16, 24):
                        emit_v_chunk(sc)
                    emit_proj(0, 5)
                    for sc in range(24, SC_ALL):
                        emit_v_chunk(sc)
                    emit_proj(0, 6)
                    emit_proj(0, 7)
                mid_fill = {}
                end_fill = {(b, qc): [] for b in range(B)
                            for qc in range(QC_B)}
            elif variant == 'bg':
                # K(b0) + the first two Q chunks up front (hc-outer so the
                # PE tracks the xT DMA); everything else -- remaining Q/K
                # projections and all V chunks -- is emitted ONCE at
                # background priority, in rough consumption order. The Tile
                # scheduler then runs it in PE idle slots, and data
                # dependencies pull each piece in just-in-time.
                emit_proj_hc_outer([(1, 0, "sT"), (0, 0, "aux")])
                emit_proj_hc_outer([(1, 1, "sT"), (1, 2, "pv"),
                                    (1, 3, "pv"), (0, 1, "aux")])
                for sc in range(8):
                    emit_v_chunk(sc)
                with tc.high_priority(offset=-1_000_000):
                    for sc in range(8, SC_B):
                        emit_v_chunk(sc)
                    for sc in range(4, HC):
                        emit_proj(1, sc)
                    emit_proj(0, 2)
                    emit_proj(0, 3)
                    for sc in range(SC_B, SC_ALL):
                        emit_v_chunk(sc)
                    for sc in range(4, HC):
                        emit_proj(0, sc)
                mid_fill = {}
                end_fill = {(b, qc): [] for b in range(B)
                            for qc in range(QC_B)}
            elif variant == 'midfill':
                # startup: K(b0) fully (every attention chunk of b0 needs
                # all of K), Q chunk 0, and the first V chunks. Everything
                # else fills PE slack inside attention chunks via mid_fill:
                # mid_fill[(b,qc)][kc] = thunks after that kc iteration,
                # paced ~1 V chunk (or 1/2 proj tile) per iteration, with a
                # >=3-iteration lead on the consuming PV.
                emit_proj_hc_outer([(1, 0, "sT"), (1, 1, "sT"),
                                    (1, 2, "pv"), (1, 3, "pv"),
                                    (0, 0, "aux")])
                for sc in range(4):
                    emit_v_chunk(sc)
                mid_fill = {
                    (0, 0): {**{kc: [V(3 + kc)] for kc in range(1, 13)},
                             13: [P(0, 1)]},
                    (0, 1): {2: [P(1, 4)], 7: [P(0, 2)], 12: [P(1, 5)]},
                    (0, 2): {2: [P(1, 6)], 7: [P(0, 3)], 12: [P(1, 7)]},
                    (0, 3): {**{kc: [V(14 + kc)] for kc in range(2, 10)},
                             11: [P(0, 4)]},
                    (1, 0): {**{kc: [V(23 + kc)] for kc in range(1, 9)}},
                }
                end_fill = {
                    (0, 0): [], (0, 1): [], (0, 2): [], (0, 3): [],
                    (1, 0): [P(0, 5)], (1, 1): [P(0, 6)],
                    (1, 2): [P(0, 7)], (1, 3): [],
                }
            else:  # 'upfront'
                emit_proj_hc_outer([(1, 0, "sT"), (1, 1, "sT"),
                                    (1, 2, "pv"), (1, 3, "pv"),
                                    (0, 0, "aux"), (0, 1, "aux")])
                for sc in range(SC_B):
                    emit_v_chunk(sc)

                def b1_slice(i):
                    t = i % 2
                    sc = 4 + 2 * (i // 2)
                    for s in (sc, sc + 1):
                        emit_proj(t, s)
                    for s in range(SC_B + 4 * i, SC_B + 4 * i + 4):
                        emit_v_chunk(s)

                mid_fill = {}
                end_fill = {
                    (0, 0): [P(0, 2), lambda: b1_slice(0)],
                    (0, 1): [P(0, 3), lambda: b1_slice(1)],
                    (0, 2): [lambda: b1_slice(2)],
                    (0, 3): [lambda: b1_slice(3)],
                    (1, 0): [], (1, 1): [], (1, 2): [], (1, 3): [],
                }

            # ---- attention ----
            for b in range(B):
                for qc in range(QC_B):  # q-chunks of 512
                    q0 = b * S + qc * 512
                    pv = [
                        psum.tile([65, 512], f32, tag="pv", bufs=2,
                                  name=f"pv_{b}_{qc}_{h}")
                        for h in range(HPC)
                    ]
                    # kc loop, software-pipelined: PV trails one iteration
                    # so the PE always issues the next S^T (which feeds the
                    # ScalarE exp stream, the pacer) before the current PV.
                    eTs = {}

                    def emit_pv(kc):
                        eT_prev = eTs.pop(kc)
                        for h in range(HPC):
                            nc.tensor.matmul(
                                pv[h],
                                v_sb[:, h, b * SC_B + kc, :],
                                eT_prev[:, h * 512:(h + 1) * 512],
                                start=(kc == 0), stop=(kc == SC_B - 1),
                            )

                    for kc in range(SC_B):  # k-chunks of 128
                        k0 = b * S + kc * 128
                        sT = psum.tile([128, 1024], f32, tag="sT", bufs=2,
                                       name=f"sT_{b}_{qc}_{kc}")
                        for h in range(HPC):
                            r0, r1 = h * 64, (h + 1) * 64
                            nc.tensor.matmul(
                                sT[:, h * 512:(h + 1) * 512],
                                qkT[r0:r1, 1, k0:k0 + 128],
                                qkT[r0:r1, 0, q0:q0 + 512],
                                start=True, stop=True,
                            )
                        eT = work.tile([128, 1024], fp16, tag="eT", bufs=3,
                                       name=f"eT_{b}_{qc}_{kc}")
                        nc.scalar.activation(
                            eT, sT, mybir.ActivationFunctionType.Exp,
                            bias=0.0, scale=SCALE,
                        )
                        eTs[kc] = eT
                        if kc > 0:
                            emit_pv(kc - 1)
                        for f in mid_fill.get((b, qc), {}).get(kc, ()):
                            f()
                    emit_pv(SC_B - 1)
                    for h in range(HPC):
                        nc.vector.tensor_copy(
                            cuT[:, 2 * b + h, qc * 512:(qc + 1) * 512], pv[h]
                        )

                    # epilogue for this chunk: transpose, normalize, bias,
                    # store. Mid-band priority: deferred into PE idle slots,
                    # but ahead of the projection/V background.
                    ep_ctx = tc.high_priority(offset=-500_000) \
                        if variant in ('bg', 'bgpaced') \
                        or variant.startswith('v3') else None
                    if ep_ctx is not None:
                        ep_ctx.__enter__()
                    for qt in range(qc * 4, qc * 4 + 4):  # q-tiles of 128
                        o_sb = work.tile([128, CPC], f32, tag="osb", bufs=3,
                                         name=f"osb_{b}_{qt}")
                        for h in range(HPC):
                            tr = psum.tile([128, 65], f32, tag="aux", bufs=2,
                                           name=f"tr_{b}_{qt}_{h}")
                            nc.tensor.transpose(
                                tr, cuT[:, 2 * b + h, qt * 128:(qt + 1) * 128],
                                ident,
                            )
                            rec = work.tile([128, 1], f32, tag="rec", bufs=4,
                                            name=f"rec_{b}_{qt}_{h}")
                            nc.vector.reciprocal(rec, tr[:, 64:65])
                            nc.vector.scalar_tensor_tensor(
                                o_sb[:, h * 64:(h + 1) * 64],
                                tr[:, 0:64], rec, bv_sb[:, h * 64:(h + 1) * 64],
                                op0=mybir.AluOpType.mult,
                                op1=mybir.AluOpType.add,
                            )
                        r0 = b * S + qt * 128
                        nc.sync.dma_start(out=out[r0:r0 + 128, :], in_=o_sb)
                    if ep_ctx is not None:
                        ep_ctx.__exit__(None, None, None)
                    for f in end_fill[(b, qc)]:
                        f()


def _emit_v4_body(nc, tc, bass, mybir, psum, work, big,
                  xT_sb, w_sb, qkT, v_sb, cuT,
                  bq_sb, bk_sb, bv_sb, ident, out):
    """Fully interleaved foreground schedule: projection/V fill work is
    metered into the attention kc-slot stream in consumption order (so the
    list scheduler's greedy choices, engine-queue order and PSUM ring reuse
    order all match the intended execution order), PV trails the exp stream
    by a per-qc lag (eT ring is deep enough to cover it), and each qc's
    epilogue is emitted inline right after its last PV."""
    fp16 = mybir.dt.float16
    f32 = mybir.dt.float32
    ET_BUFS = 18

    # ---------- PE warm-up ----------
    # The first projections trickle in at DMA pace (one hc chunk / ~1.5us),
    # which leaves the PE p-state (HAM clock gate) cold for the whole
    # startup. A chain of junk matmuls on the identity tile keeps the PE
    # continuously busy from t~0 so the real matmuls run at full clock.
    # They cycle one "sT"-ring slot; all complete before the first real S^T.
    warm = psum.tile([65, 65], f32, tag="sT", bufs=2, name="warm")
    for _ in range(24):
        nc.tensor.matmul(warm, ident, ident, start=True, stop=True,
                         skip_group_check=True)

    # ---------- upfront foreground: K(b0) sc0 + Q(b0) sc0, hc-outer ----------
    jobs = [(1, 0, "sT"), (0, 0, "aux")]
    tiles = [
        psum.tile([128, 512], f32, tag=tag, bufs=2, name=f"pj0_{t}_{sc}")
        for t, sc, tag in jobs
    ]
    for hc in range(HC):
        for (t, sc, _), ps in zip(jobs, tiles):
            nc.tensor.matmul(
                ps, w_sb[:, hc, t, :],
                xT_sb[:, hc, sc * 512:(sc + 1) * 512],
                start=(hc == 0), stop=(hc == HC - 1))
    for (t, sc, _), ps in zip(jobs, tiles):
        nc.vector.tensor_scalar_add(
            qkT[:, t, sc * 512:(sc + 1) * 512], ps,
            bq_sb if t == 0 else bk_sb)

    # ---------- fill inventory (thunks, consumption order) ----------
    PJ_COST, V_COST = 213, 53

    def proj_slices(t, sc):
        box = {}

        def mk(hc):
            def th():
                if hc == 0:
                    box['ps'] = psum.tile([128, 512], f32, tag="aux",
                                          bufs=2, name=f"pj_{t}_{sc}")
                ps = box['ps']
                nc.tensor.matmul(
                    ps, w_sb[:, hc, t, :],
                    xT_sb[:, hc, sc * 512:(sc + 1) * 512],
                    start=(hc == 0), stop=(hc == HC - 1))
                if hc == HC - 1:
                    nc.vector.tensor_scalar_add(
                        qkT[:, t, sc * 512:(sc + 1) * 512], ps,
                        bq_sb if t == 0 else bk_sb)
            return th
        return [(PJ_COST, mk(hc)) for hc in range(HC)]

    def v_slices(sc):
        box = {}

        def mk(hc):
            def th():
                if hc == 0:
                    box['ps'] = psum.tile([128, CPC], f32, tag="aux",
                                          bufs=2, name=f"psv_{sc}")
                ps = box['ps']
                nc.tensor.matmul(
                    ps, xT_sb[:, hc, sc * 128:(sc + 1) * 128],
                    w_sb[:, hc, 2, :],
                    start=(hc == 0), stop=(hc == HC - 1))
                if hc == HC - 1:
                    nc.vector.tensor_copy(
                        v_sb[:, :, sc, 0:64],
                        ps.rearrange("p (h d) -> p h d", h=HPC))
            return th
        return [(V_COST, mk(hc)) for hc in range(HC)]

    def interleave(*lists):
        res = []
        idx = [0] * len(lists)
        while any(i < len(l) for i, l in zip(idx, lists)):
            for j, l in enumerate(lists):
                if idx[j] < len(l):
                    res.append(l[idx[j]])
                    idx[j] += 1
        return res

    fills = []
    # consumed during qc0(b0): K sc1-3 + V0-3 (V stays ahead of lagged PV)
    fills += interleave(proj_slices(1, 1), v_slices(0))
    fills += interleave(proj_slices(1, 2), v_slices(1))
    fills += interleave(proj_slices(1, 3), v_slices(2))
    fills += interleave(proj_slices(0, 1), v_slices(3))
    # consumed during qc1-3(b0)
    fills += interleave(proj_slices(0, 2), v_slices(4))
    fills += v_slices(5) + v_slices(6)
    fills += interleave(proj_slices(0, 3), v_slices(7))
    fills += v_slices(8) + v_slices(9)
    fills += interleave(proj_slices(1, 4), v_slices(10))
    fills += interleave(proj_slices(1, 5), v_slices(11))
    fills += interleave(proj_slices(1, 6), v_slices(12))
    fills += interleave(proj_slices(1, 7), v_slices(13))
    fills += interleave(proj_slices(0, 4), v_slices(14))
    fills += v_slices(15)
    # consumed during b1
    fills += interleave(proj_slices(0, 5), v_slices(16))
    fills += v_slices(17) + v_slices(18)
    fills += interleave(proj_slices(0, 6), v_slices(19))
    fills += v_slices(20) + v_slices(21)
    fills += interleave(proj_slices(0, 7), v_slices(22))
    for sc in range(23, SC_ALL):
        fills += v_slices(sc)

    total_cost = sum(c for c, _ in fills)
    n_slots = B * QC_B * SC_B
    budget_per_slot = total_cost / n_slots
    fill_i = 0
    credit = 0.0

    def drain_fills(amount):
        nonlocal fill_i, credit
        credit += amount
        while fill_i < len(fills) and credit > 0:
            c, th = fills[fill_i]
            th()
            credit -= c
            fill_i += 1

    # ---------- attention stream ----------
    # PVs and per-qc closures (cuT copy + epilogue) are spliced into the
    # slot stream via a due-slot event queue, so a qc's PV tail and its
    # epilogue run inside the NEXT qc's slots instead of bursting at the
    # boundary. Epilogue transposes ride the "aux" ring: their allocation
    # order in that ring then matches real execution order.
    import heapq
    LAGS = [8, 5, 5, 5, 5, 5, 5, 2]
    events = []   # heap of (due, seq, thunk)
    ev_seq = 0

    def push_ev(due, th):
        nonlocal ev_seq
        heapq.heappush(events, (due, ev_seq, th))
        ev_seq += 1

    def flush_ev(now):
        while events and events[0][0] <= now:
            heapq.heappop(events)[2]()

    def make_closure(b, qc, pv):
        def close():
            for h in range(HPC):
                nc.vector.tensor_copy(
                    cuT[:, 2 * b + h, qc * 512:(qc + 1) * 512], pv[h])
            for qt in range(qc * 4, qc * 4 + 4):
                o_sb = work.tile([128, CPC], f32, tag="osb", bufs=3,
                                 name=f"osb_{b}_{qt}")
                for h in range(HPC):
                    tr = psum.tile([128, 65], f32, tag="aux", bufs=2,
                                   name=f"tr_{b}_{qt}_{h}")
                    nc.tensor.transpose(
                        tr, cuT[:, 2 * b + h, qt * 128:(qt + 1) * 128],
                        ident)
                    rec = work.tile([128, 1], f32, tag="rec", bufs=4,
                                    name=f"rec_{b}_{qt}_{h}")
                    nc.vector.reciprocal(rec, tr[:, 64:65])
                    nc.vector.scalar_tensor_tensor(
                        o_sb[:, h * 64:(h + 1) * 64],
                        tr[:, 0:64], rec, bv_sb[:, h * 64:(h + 1) * 64],
                        op0=mybir.AluOpType.mult,
                        op1=mybir.AluOpType.add)
                r0 = b * S + qt * 128
                nc.sync.dma_start(out=out[r0:r0 + 128, :], in_=o_sb)
        return close

    gslot = 0
    for b in range(B):
        for qc in range(QC_B):
            q0 = b * S + qc * 512
            lag = LAGS[b * QC_B + qc]
            pv = [
                psum.tile([65, 512], f32, tag="pv", bufs=2,
                          name=f"pv_{b}_{qc}_{h}")
                for h in range(HPC)
            ]
            eTs = {}

            def emit_pv(kc, pv=pv, b=b, eTs=eTs):
                eT_prev = eTs.pop(kc)
                for h in range(HPC):
                    nc.tensor.matmul(
                        pv[h],
                        v_sb[:, h, b * SC_B + kc, :],
                        eT_prev[:, h * 512:(h + 1) * 512],
                        start=(kc == 0), stop=(kc == SC_B - 1))

            for kc in range(SC_B):
                k0 = b * S + kc * 128
                sT = psum.tile([128, 1024], f32, tag="sT", bufs=2,
                               name=f"sT_{b}_{qc}_{kc}")
                for h in range(HPC):
                    r0, r1 = h * 64, (h + 1) * 64
                    nc.tensor.matmul(
                        sT[:, h * 512:(h + 1) * 512],
                        qkT[r0:r1, 1, k0:k0 + 128],
                        qkT[r0:r1, 0, q0:q0 + 512],
                        start=True, stop=True)
                eT = work.tile([128, 1024], fp16, tag="eT", bufs=ET_BUFS,
                               name=f"eT_{b}_{qc}_{kc}")
                nc.scalar.activation(
                    eT, sT, mybir.ActivationFunctionType.Exp,
                    bias=0.0, scale=SCALE)
                eTs[kc] = eT
                push_ev(gslot + lag, lambda kc=kc, f=emit_pv: f(kc))
                flush_ev(gslot)
                drain_fills(budget_per_slot)
                gslot += 1

            push_ev(gslot - 1 + lag + 0.5, make_closure(b, qc, pv))

    flush_ev(1e9)
    drain_fills(1e9)


def prep_inputs(x, Wq, bq, Wk, bk, Wv, bv):
    """Host-side prep: fold the double Q projection, transpose/cast x,
    slice per-core weights."""
    x = np.asarray(x, np.float32)
    Wq = np.asarray(Wq, np.float64)
    bq = np.asarray(bq, np.float64)
    Wq2 = (Wq @ Wq).astype(np.float32)
    bq2 = (bq @ Wq + bq).astype(np.float32)
    Wk = np.asarray(Wk, np.float32)
    Wv = np.asarray(Wv, np.float32)
    bk = np.asarray(bk, np.float32)
    bv = np.asarray(bv, np.float32)

    xT = np.ascontiguousarray(x.reshape(BS, H).T).astype(np.float16)

    in_maps = []
    for c in range(N_CORES):
        lo, hi = c * CPC, (c + 1) * CPC
        in_maps.append({
            "xT": xT,
            "wq": np.ascontiguousarray(Wq2[:, lo:hi]).astype(np.float16),
            "wk": np.ascontiguousarray(Wk[:, lo:hi]).astype(np.float16),
            "wv": np.ascontiguousarray(Wv[:, lo:hi]).astype(np.float16),
            "bq": np.ascontiguousarray(bq2[lo:hi]).reshape(CPC, 1),
            "bk": np.ascontiguousarray(bk[lo:hi]).reshape(CPC, 1),
            "bv": np.ascontiguousarray(bv[lo:hi]),
        })
    return in_maps


_CACHED = {}


def kernel(x, Wq, bq, Wk, bk, Wv, bv):
    from concourse.bass_utils import run_bass_kernel_spmd

    if "nc" not in _CACHED:
        _CACHED["nc"] = build_kernel()
    nc = _CACHED["nc"]

    in_maps = prep_inputs(x, Wq, bq, Wk, bk, Wv, bv)
    res = run_bass_kernel_spmd(nc, in_maps, core_ids=list(range(N_CORES)))

    full = np.empty((BS, NH * HD), np.float32)
    for c in range(N_CORES):
        full[:, c * CPC:(c + 1) * CPC] = res.results[c]["out"]
    return full.reshape(B, S, NH * HD)


if __name__ == "__main__":
    nc = build_kernel()
    print("built ok")



# revision 30
# speedup vs baseline: 3.4719x; 1.4878x over previous
"""Self-attention (nn_Attention_85169201480320) as a distributed Bass kernel
on 8 TRN2 NeuronCores.

Reference computation (B=2, S=2048, H=1024, NH=16, HD=64):
    mixed_query = x @ Wq + bq
    query = split_heads(mixed_query @ Wq + bq)     # double-apply bug preserved
    key   = split_heads(x @ Wk + bk)
    value = split_heads(x @ Wv + bv)
    out   = softmax(q k^T / sqrt(HD)) v            # per (batch, head)

Sharding: tensor-parallel over heads — core c owns heads {2c, 2c+1}, i.e.
columns [c*128, (c+1)*128) of the QKV projections and of the output. x is
replicated (pre-transposed and cast to fp16 on host). The double Q
projection is folded on the host: query = x @ (Wq@Wq) + (bq@Wq + bq).

Per-core device graph (no collectives needed):
  - Q^T, K^T in [d, seq] layout: psum = W_chunk^T-stationary @ x^T-moving,
    bias added on VectorE during the PSUM->SBUF copy.
  - V in [seq, d] layout (stationary = x^T chunk, moving = Wv slice), with
    a constant ones column appended (column 64) so the PV matmul also
    computes the softmax row-sums. V bias is deferred to the epilogue
    (softmax rows sum to 1).
  - Attention per (batch b, q-chunk of 512, k-chunk of 128):
      S^T[k, q] for both heads via row-tiled (64-contraction) matmuls into
      one [128, 1024] PSUM tile; one ScalarE Exp (scale=1/8 folded) into an
      fp16 E^T tile; PV matmuls accumulate [65, 512] per head over k-chunks.
  - Epilogue: PE-transpose of [65, 128] C^T tiles -> [128, 65], reciprocal
    of the sumexp column, out = C*recip + bv broadcast, DMA out.
"""

import numpy as np

B, S, H = 2, 2048, 1024
NH, HD = 16, 64
N_CORES = 8
HPC = NH // N_CORES        # heads per core = 2
CPC = HPC * HD             # output columns per core = 128
BS = B * S                 # 4096 rows total
SCALE = HD ** -0.5
# spare knob: a constant folded out of exp (cancels in the final division
# by the identically-scaled row sums); currently 0 in the exp call below.
EXP_BIAS = 2.25

HC = H // 128              # 8 contraction chunks
SC_ALL = BS // 128         # 32 seq chunks of 128
SC_B = S // 128            # 16 seq chunks per batch
QC_B = S // 512            # 4 q-chunks of 512 per batch
QT_B = S // 128            # 16 q-tiles of 128 per batch


def build_kernel(repeat: int = 1, variant: str = 'v4'):
    import concourse.bass as bass
    import concourse.mybir as mybir
    import concourse.tile as tile
    from concourse import bacc
    from concourse.masks import make_identity

    fp16 = mybir.dt.float16
    f32 = mybir.dt.float32

    nc = bacc.Bacc("TRN2", target_bir_lowering=False, debug=False,
                   num_devices=N_CORES)

    xT = nc.declare_dram_parameter("xT", [H, BS], fp16, isOutput=False)
    wq = nc.declare_dram_parameter("wq", [H, CPC], fp16, isOutput=False)
    wk = nc.declare_dram_parameter("wk", [H, CPC], fp16, isOutput=False)
    wv = nc.declare_dram_parameter("wv", [H, CPC], fp16, isOutput=False)
    bq = nc.declare_dram_parameter("bq", [CPC, 1], f32, isOutput=False)
    bk = nc.declare_dram_parameter("bk", [CPC, 1], f32, isOutput=False)
    bv = nc.declare_dram_parameter("bv", [CPC], f32, isOutput=False)
    out = nc.declare_dram_parameter("out", [BS, CPC], f32, isOutput=True)

    with tile.TileContext(nc) as tc:
        with (
            tc.tile_pool(name="big", bufs=1) as big,
            tc.tile_pool(name="work", bufs=2) as work,
            tc.tile_pool(name="psum", bufs=1, space="PSUM") as psum,
        ):
            # ---- constants / small inputs ----
            ident = big.tile([65, 65], f32)
            make_identity(nc, ident)
            ident16 = big.tile([65, 65], fp16)
            nc.vector.tensor_copy(ident16, ident)
            bq_sb = big.tile([CPC, 1], f32)
            bk_sb = big.tile([CPC, 1], f32)
            # bv broadcast to all 128 partitions: [128, 128]
            bv_sb = big.tile([128, CPC], f32)
            expb_sb = big.tile([128, 1], f32)
            nc.vector.memset(expb_sb, -EXP_BIAS)
            bv_ap = bv.ap()
            bv_bcast = bass.AP(tensor=bv_ap.tensor, offset=bv_ap.offset,
                               ap=[[0, 128], [1, CPC]])
            if not variant.startswith(('v3', 'v4')):
                nc.sync.dma_start(out=bq_sb, in_=bq[:, :])
                nc.sync.dma_start(out=bk_sb, in_=bk[:, :])
                nc.gpsimd.dma_start(out=bv_sb, in_=bv_bcast)

            # ---- big persistent SBUF tensors ----
            # weights first: the first projection matmuls need w + one xT
            # chunk, so don't queue 8MB of xT DMA ahead of them.
            w_sb = big.tile([128, HC, 3, CPC], fp16)       # 6KB/part
            xT_sb = big.tile([128, HC, BS], fp16)          # 64KB/part
            if variant.startswith(('v3', 'v4')):
                # DMA issue costs ~650ns on the issuing sequencer, so the
                # serial-on-SP baseline pays ~19us before the first
                # projection data is even queued. Spread issues over four
                # engine rings (SP, Pool, DVE, PE -- ACT stays free for the
                # exp stream) and use 2-hc strided chunks so the first 512
                # seq-columns of every hc land within ~2.5us.
                # wq/wk first (block the first projection), then the first
                # 512 seq-cols of every hc, then wv (first V fill ~10us),
                # then the rest in consumption order. bv does not matter
                # until the first epilogue (~30us) -- it goes last.
                for t, w, eng in ((0, wq, nc.gpsimd), (1, wk, nc.scalar)):
                    eng.dma_start(
                        out=w_sb[:, :, t, :],
                        in_=w.ap().rearrange("(c p) m -> p c m", p=128))
                ph0 = (nc.sync, nc.gpsimd, nc.scalar, nc.sync)
                for i in range(4):
                    ph0[i].dma_start(
                        out=xT_sb[:, 2 * i:2 * i + 2, 0:512],
                        in_=xT[256 * i:256 * i + 256, 0:512].rearrange(
                            "(c p) m -> p c m", p=128))
                nc.gpsimd.dma_start(
                    out=w_sb[:, :, 2, :],
                    in_=wv.ap().rearrange("(c p) m -> p c m", p=128))
                nc.scalar.dma_start(out=bq_sb, in_=bq[:, :])
                nc.scalar.dma_start(out=bk_sb, in_=bk[:, :])
                for c0, c1 in ((512, 1024), (1024, 1536), (1536, 2048),
                               (2048, 3072), (3072, 4096)):
                    for i in range(4):
                        (nc.sync if i % 2 == 0 else nc.gpsimd).dma_start(
                            out=xT_sb[:, 2 * i:2 * i + 2, c0:c1],
                            in_=xT[256 * i:256 * i + 256, c0:c1].rearrange(
                                "(c p) m -> p c m", p=128))
                nc.gpsimd.dma_start(out=bv_sb, in_=bv_bcast)
            else:
                # one strided DMA per weight tensor:
                # [1024,128] -> [128, hc, 128]
                for t, w in ((0, wq), (1, wk), (2, wv)):
                    nc.sync.dma_start(
                        out=w_sb[:, :, t, :],
                        in_=w.ap().rearrange("(c p) m -> p c m", p=128))
                # on the SP ring (NOT the ACT ring: the ACT sequencer must be
                # free to issue the first exp the moment S^T(0) lands -- DMA
                # issues cost ~0.7-2.2us each on the issuing sequencer).
                # Order: the first 512 seq-columns of every hc chunk land
                # first (that is all K/Q chunk 0 needs, so the exp stream can
                # start ~10us earlier), then the rest of batch 0, then b1.
                for hc in range(HC):
                    nc.sync.dma_start(
                        out=xT_sb[:, hc, 0:512],
                        in_=xT[hc * 128:(hc + 1) * 128, 0:512])
                for hc in range(HC):
                    nc.sync.dma_start(
                        out=xT_sb[:, hc, 512:S],
                        in_=xT[hc * 128:(hc + 1) * 128, 512:S])
                for hc in range(HC):
                    nc.sync.dma_start(
                        out=xT_sb[:, hc, S:BS],
                        in_=xT[hc * 128:(hc + 1) * 128, S:BS])

            qkT = big.tile([128, 2, BS], fp16)             # 16KB/part
            v_sb = big.tile([128, HPC, SC_ALL, 65], fp16)  # 8.3KB/part
            cuT = big.tile([65, 2 * HPC, S], fp16)         # 16KB/part

            # ones column of V_aug (written once; V copies touch only 0:64)
            nc.vector.memset(v_sb[:, :, :, 64:65], 1.0)

            if variant.startswith(('v3', 'v4')):
                # dummy exp: pulls the ~2.7us ACT table load into the DMA
                # phase, where the ACT engine is otherwise idle.
                dmy = big.tile([1, 2], f32)
                nc.vector.memset(dmy, 0.0)
                dmy_o = big.tile([1, 2], fp16)
                nc.scalar.activation(
                    dmy_o, dmy, mybir.ActivationFunctionType.Exp,
                    bias=0.0, scale=1.0)

            def emit_body():
                if variant.startswith('v4'):
                    _emit_v4_body(nc, tc, bass, mybir, psum, work, big,
                                  xT_sb, w_sb, qkT, v_sb, cuT,
                                  bq_sb, bk_sb, bv_sb, ident, out,
                                  ident16)
                else:
                    _emit_attention_body(nc, tc, bass, mybir, psum, work,
                                         big, xT_sb, w_sb, qkT, v_sb, cuT,
                                         bq_sb, bk_sb, bv_sb, ident, out,
                                         expb_sb, variant)

            if repeat == 1:
                emit_body()
            else:
                with tc.For_i(0, repeat, 1):
                    emit_body()

    nc.finalize()
    return nc


def _emit_attention_body(nc, tc, bass, mybir, psum, work, big,
                         xT_sb, w_sb, qkT, v_sb, cuT,
                         bq_sb, bk_sb, bv_sb, ident, out, expb_sb,
                         variant='bg'):
            fp16 = mybir.dt.float16
            f32 = mybir.dt.float32
            # ---- emission plan ----
            # attention chunk (b, qc) needs: Q chunk sc=4b+qc, ALL of K for
            # batch b, and V chunks racing ahead of its kc loop. So: project
            # K(b0) + Q(b0,sc0) first (hc-outer, so the PE starts on the
            # first 512KB xT DMA), start attention immediately, and feed the
            # remaining Q/K/V projections in as fillers between (and inside)
            # attention chunks, where they soak up PE slack under the
            # ScalarE-paced exp stream.
            def emit_proj_hc_outer(jobs):
                # jobs: list of (t, sc, tag) -> one [128,512] psum tile each
                tiles = [
                    psum.tile([128, 512], f32, tag=tag, bufs=2,
                              name=f"pj0_{t}_{sc}")
                    for t, sc, tag in jobs
                ]
                for hc in range(HC):
                    for (t, sc, _), ps in zip(jobs, tiles):
                        nc.tensor.matmul(
                            ps,
                            w_sb[:, hc, t, :],
                            xT_sb[:, hc, sc * 512:(sc + 1) * 512],
                            start=(hc == 0), stop=(hc == HC - 1),
                        )
                for (t, sc, _), ps in zip(jobs, tiles):
                    nc.vector.tensor_scalar_add(
                        qkT[:, t, sc * 512:(sc + 1) * 512], ps,
                        bq_sb if t == 0 else bk_sb,
                    )

            def emit_proj(t, sc):
                ps = psum.tile([128, 512], f32, tag="aux", bufs=2,
                               name=f"pj_{t}_{sc}")
                for hc in range(HC):
                    nc.tensor.matmul(
                        ps,
                        w_sb[:, hc, t, :],
                        xT_sb[:, hc, sc * 512:(sc + 1) * 512],
                        start=(hc == 0), stop=(hc == HC - 1),
                    )
                nc.vector.tensor_scalar_add(
                    qkT[:, t, sc * 512:(sc + 1) * 512], ps,
                    bq_sb if t == 0 else bk_sb,
                )

            def emit_v_chunk(sc):
                ps = psum.tile([128, CPC], f32, tag="aux", bufs=2,
                               name=f"psv_{sc}")
                for hc in range(HC):
                    nc.tensor.matmul(
                        ps,
                        xT_sb[:, hc, sc * 128:(sc + 1) * 128],
                        w_sb[:, hc, 2, :],
                        start=(hc == 0), stop=(hc == HC - 1),
                    )
                # [128, 2, 64] strided copy into v_sb (both heads)
                nc.vector.tensor_copy(
                    v_sb[:, :, sc, 0:64],
                    ps.rearrange("p (h d) -> p h d", h=HPC),
                )

            V = lambda s: (lambda: emit_v_chunk(s))
            P = lambda t, s: (lambda: emit_proj(t, s))

            if variant == 'bgpaced':
                # like 'bg', but gives the scheduler a pacing hint per
                # background piece (earliest useful time, us) so it does not
                # front-stuff V work ahead of the first exp stream.
                emit_proj_hc_outer([(1, 0, "sT"), (1, 1, "sT"),
                                    (1, 2, "pv"), (1, 3, "pv"),
                                    (0, 0, "aux"), (0, 1, "aux")])
                with tc.high_priority(offset=-1_000_000):
                    def at(us, f):
                        with tc.tile_wait_until(us / 1000.0):
                            f()
                    for sc in range(SC_B):
                        at(10 + sc * 1.2, lambda s=sc: emit_v_chunk(s))
                    at(28, lambda: emit_proj(0, 2))
                    at(38, lambda: emit_proj(0, 3))
                    for i, sc in enumerate(range(4, HC)):
                        at(45 + 6 * i, lambda s=sc: emit_proj(1, s))
                    for sc in range(SC_B, SC_ALL):
                        at(55 + (sc - SC_B) * 1.2, lambda s=sc: emit_v_chunk(s))
                    at(72, lambda: emit_proj(0, 4))
                    at(88, lambda: emit_proj(0, 5))
                    at(107, lambda: emit_proj(0, 6))
                    at(126, lambda: emit_proj(0, 7))
                mid_fill = {}
                end_fill = {(b, qc): [] for b in range(B)
                            for qc in range(QC_B)}
            elif variant.startswith('v3'):
                # K(b0) sc0 + Q(b0) sc0 foreground (hc-outer, DMA-paced);
                # everything else background in exact consumption order so
                # the greedy scheduler's ready-queue matches the exp
                # stream's needs and data-arrival order.
                emit_proj_hc_outer([(1, 0, "sT"), (0, 0, "aux")])
                with tc.high_priority(offset=-1_000_000):
                    emit_proj(1, 1)
                    for sc in range(0, 4):
                        emit_v_chunk(sc)
                    emit_proj(1, 2)
                    for sc in range(4, 8):
                        emit_v_chunk(sc)
                    emit_proj(1, 3)
                    for sc in range(8, 12):
                        emit_v_chunk(sc)
                    emit_proj(0, 1)
                    for sc in range(12, 16):
                        emit_v_chunk(sc)
                    emit_proj(0, 2)
                    emit_proj(0, 3)
                    for sc in range(4, HC):   # K(b1)
                        emit_proj(1, sc)
                    emit_proj(0, 4)           # Q(b1, qc0)
                    for sc in range(16, 24):
                        emit_v_chunk(sc)
                    emit_proj(0, 5)
                    for sc in range(24, SC_ALL):
                        emit_v_chunk(sc)
                    emit_proj(0, 6)
                    emit_proj(0, 7)
                mid_fill = {}
                end_fill = {(b, qc): [] for b in range(B)
                            for qc in range(QC_B)}
            elif variant == 'bg':
                # K(b0) + the first two Q chunks up front (hc-outer so the
                # PE tracks the xT DMA); everything else -- remaining Q/K
                # projections and all V chunks -- is emitted ONCE at
                # background priority, in rough consumption order. The Tile
                # scheduler then runs it in PE idle slots, and data
                # dependencies pull each piece in just-in-time.
                emit_proj_hc_outer([(1, 0, "sT"), (0, 0, "aux")])
                emit_proj_hc_outer([(1, 1, "sT"), (1, 2, "pv"),
                                    (1, 3, "pv"), (0, 1, "aux")])
                for sc in range(8):
                    emit_v_chunk(sc)
                with tc.high_priority(offset=-1_000_000):
                    for sc in range(8, SC_B):
                        emit_v_chunk(sc)
                    for sc in range(4, HC):
                        emit_proj(1, sc)
                    emit_proj(0, 2)
                    emit_proj(0, 3)
                    for sc in range(SC_B, SC_ALL):
                        emit_v_chunk(sc)
                    for sc in range(4, HC):
                        emit_proj(0, sc)
                mid_fill = {}
                end_fill = {(b, qc): [] for b in range(B)
                            for qc in range(QC_B)}
            elif variant == 'midfill':
                # startup: K(b0) fully (every attention chunk of b0 needs
                # all of K), Q chunk 0, and the first V chunks. Everything
                # else fills PE slack inside attention chunks via mid_fill:
                # mid_fill[(b,qc)][kc] = thunks after that kc iteration,
                # paced ~1 V chunk (or 1/2 proj tile) per iteration, with a
                # >=3-iteration lead on the consuming PV.
                emit_proj_hc_outer([(1, 0, "sT"), (1, 1, "sT"),
                                    (1, 2, "pv"), (1, 3, "pv"),
                                    (0, 0, "aux")])
                for sc in range(4):
                    emit_v_chunk(sc)
                mid_fill = {
                    (0, 0): {**{kc: [V(3 + kc)] for kc in range(1, 13)},
                             13: [P(0, 1)]},
                    (0, 1): {2: [P(1, 4)], 7: [P(0, 2)], 12: [P(1, 5)]},
                    (0, 2): {2: [P(1, 6)], 7: [P(0, 3)], 12: [P(1, 7)]},
                    (0, 3): {**{kc: [V(14 + kc)] for kc in range(2, 10)},
                             11: [P(0, 4)]},
                    (1, 0): {**{kc: [V(23 + kc)] for kc in range(1, 9)}},
                }
                end_fill = {
                    (0, 0): [], (0, 1): [], (0, 2): [], (0, 3): [],
                    (1, 0): [P(0, 5)], (1, 1): [P(0, 6)],
                    (1, 2): [P(0, 7)], (1, 3): [],
                }
            else:  # 'upfront'
                emit_proj_hc_outer([(1, 0, "sT"), (1, 1, "sT"),
                                    (1, 2, "pv"), (1, 3, "pv"),
                                    (0, 0, "aux"), (0, 1, "aux")])
                for sc in range(SC_B):
                    emit_v_chunk(sc)

                def b1_slice(i):
                    t = i % 2
                    sc = 4 + 2 * (i // 2)
                    for s in (sc, sc + 1):
                        emit_proj(t, s)
                    for s in range(SC_B + 4 * i, SC_B + 4 * i + 4):
                        emit_v_chunk(s)

                mid_fill = {}
                end_fill = {
                    (0, 0): [P(0, 2), lambda: b1_slice(0)],
                    (0, 1): [P(0, 3), lambda: b1_slice(1)],
                    (0, 2): [lambda: b1_slice(2)],
                    (0, 3): [lambda: b1_slice(3)],
                    (1, 0): [], (1, 1): [], (1, 2): [], (1, 3): [],
                }

            # ---- attention ----
            for b in range(B):
                for qc in range(QC_B):  # q-chunks of 512
                    q0 = b * S + qc * 512
                    pv = [
                        psum.tile([65, 512], f32, tag="pv", bufs=2,
                                  name=f"pv_{b}_{qc}_{h}")
                        for h in range(HPC)
                    ]
                    # kc loop, software-pipelined: PV trails one iteration
                    # so the PE always issues the next S^T (which feeds the
                    # ScalarE exp stream, the pacer) before the current PV.
                    eTs = {}

                    def emit_pv(kc):
                        eT_prev = eTs.pop(kc)
                        for h in range(HPC):
                            nc.tensor.matmul(
                                pv[h],
                                v_sb[:, h, b * SC_B + kc, :],
                                eT_prev[:, h * 512:(h + 1) * 512],
                                start=(kc == 0), stop=(kc == SC_B - 1),
                            )

                    for kc in range(SC_B):  # k-chunks of 128
                        k0 = b * S + kc * 128
                        sT = psum.tile([128, 1024], f32, tag="sT", bufs=2,
                                       name=f"sT_{b}_{qc}_{kc}")
                        for h in range(HPC):
                            r0, r1 = h * 64, (h + 1) * 64
                            nc.tensor.matmul(
                                sT[:, h * 512:(h + 1) * 512],
                                qkT[r0:r1, 1, k0:k0 + 128],
                                qkT[r0:r1, 0, q0:q0 + 512],
                                start=True, stop=True,
                            )
                        eT = work.tile([128, 1024], fp16, tag="eT", bufs=3,
                                       name=f"eT_{b}_{qc}_{kc}")
                        nc.scalar.activation(
                            eT, sT, mybir.ActivationFunctionType.Exp,
                            bias=0.0, scale=SCALE,
                        )
                        eTs[kc] = eT
                        if kc > 0:
                            emit_pv(kc - 1)
                        for f in mid_fill.get((b, qc), {}).get(kc, ()):
                            f()
                    emit_pv(SC_B - 1)
                    for h in range(HPC):
                        nc.vector.tensor_copy(
                            cuT[:, 2 * b + h, qc * 512:(qc + 1) * 512], pv[h]
                        )

                    # epilogue for this chunk: transpose, normalize, bias,
                    # store. Mid-band priority: deferred into PE idle slots,
                    # but ahead of the projection/V background.
                    ep_ctx = tc.high_priority(offset=-500_000) \
                        if variant in ('bg', 'bgpaced') \
                        or variant.startswith('v3') else None
                    if ep_ctx is not None:
                        ep_ctx.__enter__()
                    for qt in range(qc * 4, qc * 4 + 4):  # q-tiles of 128
                        o_sb = work.tile([128, CPC], f32, tag="osb", bufs=3,
                                         name=f"osb_{b}_{qt}")
                        for h in range(HPC):
                            tr = psum.tile([128, 65], fp16, tag="aux", bufs=2,
                                           name=f"tr_{b}_{qt}_{h}")
                            nc.tensor.transpose(
                                tr, cuT[:, 2 * b + h, qt * 128:(qt + 1) * 128],
                                ident,
                            )
                            rec = work.tile([128, 1], f32, tag="rec", bufs=4,
                                            name=f"rec_{b}_{qt}_{h}")
                            nc.vector.reciprocal(rec, tr[:, 64:65])
                            nc.vector.scalar_tensor_tensor(
                                o_sb[:, h * 64:(h + 1) * 64],
                                tr[:, 0:64], rec, bv_sb[:, h * 64:(h + 1) * 64],
                                op0=mybir.AluOpType.mult,
                                op1=mybir.AluOpType.add,
                            )
                        r0 = b * S + qt * 128
                        nc.sync.dma_start(out=out[r0:r0 + 128, :], in_=o_sb)
                    if ep_ctx is not None:
                        ep_ctx.__exit__(None, None, None)
                    for f in end_fill[(b, qc)]:
                        f()


def _emit_v4_body(nc, tc, bass, mybir, psum, work, big,
                  xT_sb, w_sb, qkT, v_sb, cuT,
                  bq_sb, bk_sb, bv_sb, ident, out, ident16):
    """Fully interleaved foreground schedule: projection/V fill work is
    metered into the attention kc-slot stream in consumption order (so the
    list scheduler's greedy choices, engine-queue order and PSUM ring reuse
    order all match the intended execution order), PV trails the exp stream
    by a per-qc lag (eT ring is deep enough to cover it), and each qc's
    epilogue is emitted inline right after its last PV."""
    fp16 = mybir.dt.float16
    f32 = mybir.dt.float32
    ET_BUFS = 18

    # ---------- PE warm-up ----------
    # The first projections trickle in at DMA pace (one hc chunk / ~1.5us),
    # which leaves the PE p-state (HAM clock gate) cold for the whole
    # startup. A chain of junk matmuls keeps the PE continuously busy from
    # t~0 so the real matmuls run at full clock. Uses a memset tile (not
    # ident, whose iota/copy chain lands at ~2.4us) so it starts instantly.
    warm_in = big.tile([64, 64], fp16)
    nc.vector.memset(warm_in, 0.5)
    warm = psum.tile([65, 65], f32, tag="sT", bufs=2, name="warm")
    for _ in range(24):
        nc.tensor.matmul(warm[0:64, 0:64], warm_in, warm_in,
                         start=True, stop=True, skip_group_check=True)

    # ---------- upfront foreground: K(b0) sc0 + Q(b0) sc0, hc-outer ----------
    jobs = [(1, 0, "sT"), (0, 0, "aux")]
    tiles = [
        psum.tile([128, 512], f32, tag=tag, bufs=2, name=f"pj0_{t}_{sc}")
        for t, sc, tag in jobs
    ]
    for hc in range(HC):
        for (t, sc, _), ps in zip(jobs, tiles):
            nc.tensor.matmul(
                ps, w_sb[:, hc, t, :],
                xT_sb[:, hc, sc * 512:(sc + 1) * 512],
                start=(hc == 0), stop=(hc == HC - 1))
    for (t, sc, _), ps in zip(jobs, tiles):
        nc.vector.tensor_scalar_add(
            qkT[:, t, sc * 512:(sc + 1) * 512], ps,
            bq_sb if t == 0 else bk_sb)

    # ---------- fill inventory (thunks, consumption order) ----------
    PJ_COST, V_COST = 213, 53

    def proj_slices(t, sc):
        box = {}

        def mk(hc):
            def th():
                if hc == 0:
                    box['ps'] = psum.tile([128, 512], f32, tag="aux",
                                          bufs=2, name=f"pj_{t}_{sc}")
                ps = box['ps']
                nc.tensor.matmul(
                    ps, w_sb[:, hc, t, :],
                    xT_sb[:, hc, sc * 512:(sc + 1) * 512],
                    start=(hc == 0), stop=(hc == HC - 1))
                if hc == HC - 1:
                    nc.vector.tensor_scalar_add(
                        qkT[:, t, sc * 512:(sc + 1) * 512], ps,
                        bq_sb if t == 0 else bk_sb)
            return th
        return [(PJ_COST, mk(hc)) for hc in range(HC)]

    def v_slices(sc):
        box = {}

        def mk(hc):
            def th():
                if hc == 0:
                    box['ps'] = psum.tile([128, CPC], f32, tag="aux",
                                          bufs=2, name=f"psv_{sc}")
                ps = box['ps']
                nc.tensor.matmul(
                    ps, xT_sb[:, hc, sc * 128:(sc + 1) * 128],
                    w_sb[:, hc, 2, :],
                    start=(hc == 0), stop=(hc == HC - 1))
                if hc == HC - 1:
                    nc.vector.tensor_copy(
                        v_sb[:, :, sc, 0:64],
                        ps.rearrange("p (h d) -> p h d", h=HPC))
            return th
        return [(V_COST, mk(hc)) for hc in range(HC)]

    # Deadline-scheduled fill emission. Each fill item gets a deadline =
    # its first consumer's kc-slot minus a margin; per slot we emit
    # everything past-deadline (correctness: Tile deps follow emission
    # order) plus a bounded lookahead so work spreads instead of bursting
    # at require() points.
    LAGS = [8, 5, 5, 5, 5, 5, 5, 2]
    MARGIN = 2.0
    LOOKAHEAD = 10.0

    def lag_of(b, qc):
        return LAGS[b * QC_B + qc]

    items = []   # (deadline, cost, thunk, key)

    def add_group(slices, deadline, key):
        n = len(slices)
        for i, (c, th) in enumerate(slices):
            d = deadline - MARGIN - (n - 1 - i) * 0.4
            items.append((d, c, th, key if i == n - 1 else None))

    for b in range(B):
        for j in range(QC_B):
            sc = b * QC_B + j
            if sc == 0:
                continue
            add_group(proj_slices(1, sc), 64 * b + 4 * j, ('P', 1, sc))
        for qc in range(QC_B):
            sc = b * QC_B + qc
            if sc == 0:
                continue
            add_group(proj_slices(0, sc), 64 * b + 16 * qc, ('P', 0, sc))
    for sc in range(SC_ALL):
        b = sc // SC_B
        kc = sc % SC_B
        add_group(v_slices(sc), 64 * b + kc + lag_of(b, 0), ('V', sc))

    items.sort(key=lambda x: x[0])
    fill_i = 0
    done_keys = {('P', 1, 0), ('P', 0, 0)}

    def _emit_one():
        nonlocal fill_i
        d, c, th, key = items[fill_i]
        th()
        fill_i += 1
        if key is not None:
            done_keys.add(key)

    def drain_to(now):
        while fill_i < len(items) and items[fill_i][0] <= now:
            _emit_one()

    def require(key):
        while fill_i < len(items) and key not in done_keys:
            _emit_one()

    # ---------- attention stream ----------
    # PVs and per-qc closures (cuT copy + epilogue) are spliced into the
    # slot stream via a due-slot event queue, so a qc's PV tail and its
    # epilogue run inside the NEXT qc's slots instead of bursting at the
    # boundary. Epilogue transposes ride the "aux" ring: their allocation
    # order in that ring then matches real execution order.
    import heapq
    LAGS = [8, 5, 5, 5, 5, 5, 5, 2]
    events = []   # heap of (due, seq, thunk)
    ev_seq = 0

    def push_ev(due, th):
        nonlocal ev_seq
        heapq.heappush(events, (due, ev_seq, th))
        ev_seq += 1

    def flush_ev(now):
        while events and events[0][0] <= now:
            heapq.heappop(events)[2]()

    def make_closure(b, qc, pv):
        last = (b == B - 1 and qc == QC_B - 1)

        def close():
            for h in range(HPC):
                # for the final qc the exp stream is over -- split the two
                # PSUM->SBUF staging copies across ACT and DVE.
                if last and h == 0:
                    nc.scalar.copy(
                        cuT[:, 2 * b + h, qc * 512:(qc + 1) * 512], pv[h])
                else:
                    nc.vector.tensor_copy(
                        cuT[:, 2 * b + h, qc * 512:(qc + 1) * 512], pv[h])
            # all 8 transposes of this qc go into ONE aux-ring slot so they
            # do not serialize on ring reuse.
            tr = psum.tile([128, 8, 66], fp16, tag="aux", bufs=2,
                           name=f"tr_{b}_{qc}")
            # all 8 transposes back-to-back on the PE first (slice-level
            # dependency tracking is coarse on this tile -- interleaving
            # reads would serialize each transpose behind the previous
            # read), then the DVE normalize chain.
            for qt in range(qc * 4, qc * 4 + 4):
                j = (qt - qc * 4) * 2
                for h in range(HPC):
                    nc.tensor.transpose(
                        tr[:, j + h, 0:65],
                        cuT[:, 2 * b + h, qt * 128:(qt + 1) * 128],
                        ident16)
            for qt in range(qc * 4, qc * 4 + 4):
                j = (qt - qc * 4) * 2
                o_sb = work.tile([128, CPC], f32, tag="osb", bufs=3,
                                 name=f"osb_{b}_{qt}")
                for h in range(HPC):
                    rec = work.tile([128, 1], f32, tag="rec", bufs=4,
                                    name=f"rec_{b}_{qt}_{h}")
                    nc.vector.reciprocal(rec, tr[:, j + h, 64:65])
                    nc.vector.scalar_tensor_tensor(
                        o_sb[:, h * 64:(h + 1) * 64],
                        tr[:, j + h, 0:64], rec,
                        bv_sb[:, h * 64:(h + 1) * 64],
                        op0=mybir.AluOpType.mult,
                        op1=mybir.AluOpType.add)
                r0 = b * S + qt * 128
                nc.sync.dma_start(out=out[r0:r0 + 128, :], in_=o_sb)
        return close

    gslot = 0
    for b in range(B):
        for qc in range(QC_B):
            q0 = b * S + qc * 512
            lag = LAGS[b * QC_B + qc]
            pv = [
                psum.tile([65, 512], f32, tag="pv", bufs=2,
                          name=f"pv_{b}_{qc}_{h}")
                for h in range(HPC)
            ]
            eTs = {}

            def emit_pv(kc, pv=pv, b=b, eTs=eTs):
                require(('V', b * SC_B + kc))
                eT_prev = eTs.pop(kc)
                for h in range(HPC):
                    nc.tensor.matmul(
                        pv[h],
                        v_sb[:, h, b * SC_B + kc, :],
                        eT_prev[:, h * 512:(h + 1) * 512],
                        start=(kc == 0), stop=(kc == SC_B - 1))

            for kc in range(SC_B):
                require(('P', 1, b * QC_B + kc // 4))
                require(('P', 0, b * QC_B + qc))
                k0 = b * S + kc * 128
                sT = psum.tile([128, 1024], f32, tag="sT", bufs=2,
                               name=f"sT_{b}_{qc}_{kc}")
                for h in range(HPC):
                    r0, r1 = h * 64, (h + 1) * 64
                    nc.tensor.matmul(
                        sT[:, h * 512:(h + 1) * 512],
                        qkT[r0:r1, 1, k0:k0 + 128],
                        qkT[r0:r1, 0, q0:q0 + 512],
                        start=True, stop=True)
                eT = work.tile([128, 1024], fp16, tag="eT", bufs=ET_BUFS,
                               name=f"eT_{b}_{qc}_{kc}")
                nc.scalar.activation(
                    eT, sT, mybir.ActivationFunctionType.Exp,
                    bias=0.0, scale=SCALE)
                eTs[kc] = eT
                push_ev(gslot + lag, lambda kc=kc, f=emit_pv: f(kc))
                flush_ev(gslot)
                drain_to(gslot + LOOKAHEAD)
                gslot += 1

            push_ev(gslot - 1 + lag + 0.5, make_closure(b, qc, pv))

    flush_ev(1e9)
    drain_to(1e9)


def prep_inputs(x, Wq, bq, Wk, bk, Wv, bv):
    """Host-side prep: fold the double Q projection, transpose/cast x,
    slice per-core weights."""
    x = np.asarray(x, np.float32)
    Wq = np.asarray(Wq, np.float64)
    bq = np.asarray(bq, np.float64)
    Wq2 = (Wq @ Wq).astype(np.float32)
    bq2 = (bq @ Wq + bq).astype(np.float32)
    Wk = np.asarray(Wk, np.float32)
    Wv = np.asarray(Wv, np.float32)
    bk = np.asarray(bk, np.float32)
    bv = np.asarray(bv, np.float32)

    xT = np.ascontiguousarray(x.reshape(BS, H).T).astype(np.float16)

    in_maps = []
    for c in range(N_CORES):
        lo, hi = c * CPC, (c + 1) * CPC
        in_maps.append({
            "xT": xT,
            "wq": np.ascontiguousarray(Wq2[:, lo:hi]).astype(np.float16),
            "wk": np.ascontiguousarray(Wk[:, lo:hi]).astype(np.float16),
            "wv": np.ascontiguousarray(Wv[:, lo:hi]).astype(np.float16),
            "bq": np.ascontiguousarray(bq2[lo:hi]).reshape(CPC, 1),
            "bk": np.ascontiguousarray(bk[lo:hi]).reshape(CPC, 1),
            "bv": np.ascontiguousarray(bv[lo:hi]),
        })
    return in_maps


_CACHED = {}


def kernel(x, Wq, bq, Wk, bk, Wv, bv):
    from concourse.bass_utils import run_bass_kernel_spmd

    if "nc" not in _CACHED:
        _CACHED["nc"] = build_kernel()
    nc = _CACHED["nc"]

    in_maps = prep_inputs(x, Wq, bq, Wk, bk, Wv, bv)
    res = run_bass_kernel_spmd(nc, in_maps, core_ids=list(range(N_CORES)))

    full = np.empty((BS, NH * HD), np.float32)
    for c in range(N_CORES):
        full[:, c * CPC:(c + 1) * CPC] = res.results[c]["out"]
    return full.reshape(B, S, NH * HD)


if __name__ == "__main__":
    nc = build_kernel()
    print("built ok")

